# revision 2
# baseline (speedup 1.0000x reference)
"""Causal attention (nn_CausalAttention) TRN2 Bass kernel.

Strategy: tensor-parallel over the 16 heads -> 2 heads per NeuronCore.
Each core:
  - computes its 128 columns of the qkv projection directly in transposed
    (col-major) layout: M_s = W_s^T @ x^T accumulated over d-chunks on the PE
  - scatters M_s (stride-3 in t) into per-head-pair qT/kT/vT SBUF tiles,
    reproducing the reference's raw-memory reshape(3,B,T,HS,NH) semantics
  - runs causal attention per head with scores kept transposed
    (tk on partitions): sT = kT_blk^T @ qT_tile, exp on ScalarE (scale folded),
    A@V accumulated in PSUM with an appended ones-column producing row sums,
  - transposes ctx back on the PE, normalizes with DVE reciprocal, DMAs out.
All matmuls use float32r (fp32 with 11-bit mantissa, full-rate on the PE).
"""

import os
import sys

import numpy as np

for _p in ("/opt/trn_rl_repo", "/root/.axon_site/_ro/trn_rl_repo"):
    if os.path.isdir(_p) and _p not in sys.path:
        sys.path.insert(0, _p)

import concourse.bass as bass  # noqa: E402
import concourse.mybir as mybir  # noqa: E402
import concourse.tile as tile  # noqa: E402
from concourse import bacc  # noqa: E402
from concourse.bass_utils import run_bass_kernel_spmd  # noqa: E402
from concourse.masks import make_identity  # noqa: E402

f32 = mybir.dt.float32
f32r = mybir.dt.float32r
AF = mybir.ActivationFunctionType

T, D, NH, HS = 4096, 1024, 16, 64
SCALE = HS ** -0.5  # 0.125
NCORES = 8
TQ = 512
NTQ = T // TQ  # 8
TK = 128
NTK = T // TK  # 32
U3 = 1366
QKV_W = U3 * 3  # 4098

# (u_lo, u_hi, dst, t0): M_s[:, u] for u in [u_lo,u_hi) -> dst[:, 3*(u-u_lo)+t0]
SPLITS = {
    0: [(0, 1366, "q", 0), (1366, 2731, "k", 2), (2731, 4096, "v", 1)],
    1: [(0, 1365, "q", 1), (1365, 2731, "k", 0), (2731, 4096, "v", 2)],
    2: [(0, 1365, "q", 2), (1365, 2730, "k", 1), (2730, 4096, "v", 0)],
}


def _build_program():
    nc = bacc.Bacc(
        "TRN2", target_bir_lowering=False, debug=False, num_devices=NCORES
    )
    xt_d = nc.dram_tensor("xt", [D, T], f32r, kind="ExternalInput").ap()
    w_d = nc.dram_tensor("w", [3, D, 128], f32r, kind="ExternalInput").ap()
    b_d = nc.dram_tensor("b", [128, 3], f32, kind="ExternalInput").ap()
    m_d = nc.dram_tensor("m", [128, 128], f32r, kind="ExternalInput").ap()
    m2_d = nc.dram_tensor("m2", [128, 256], f32r, kind="ExternalInput").ap()
    out_d = nc.dram_tensor("out", [T, 128], f32, kind="ExternalOutput").ap()

    with tile.TileContext(nc) as tc:
        with (
            tc.tile_pool(name="const", bufs=1) as const_pool,
            tc.tile_pool(name="big", bufs=1) as big_pool,
        ):
            # constants
            w_all = const_pool.tile([128, 3 * 8 * 128], f32r, tag="w_all")
            w_view = w_d.rearrange("s (dc d) c -> d (s dc) c", d=128)
            nc.sync.dma_start(
                out=w_all[:].rearrange("d (g c) -> d g c", c=128), in_=w_view
            )
            b_sb = const_pool.tile([128, 3], f32, tag="b_sb")
            nc.sync.dma_start(out=b_sb[:], in_=b_d[:])
            mask_sb = const_pool.tile([128, 128], f32r, tag="mask")
            nc.sync.dma_start(out=mask_sb[:], in_=m_d[:])
            mask2_sb = const_pool.tile([128, 256], f32r, tag="mask2")
            nc.sync.dma_start(out=mask2_sb[:], in_=m2_d[:])
            ident = const_pool.tile([128, 128], f32, tag="ident")
            make_identity(nc, ident[:])
            ones_sb = const_pool.tile([128, 1], f32, tag="ones")
            nc.vector.memset(ones_sb[:], 1.0)

            qT = big_pool.tile([128, QKV_W], f32r, tag="qT")
            kT = big_pool.tile([128, QKV_W], f32r, tag="kT")
            vT = big_pool.tile([128, QKV_W], f32, tag="vT")
            vext = [
                big_pool.tile([128, 65 * NTK], f32r, tag=f"vext{h}", name=f"vext{h}") for h in (0, 1)
            ]
            dstmap = {"q": qT, "k": kT, "v": vT}

            # ---- Phase A: qkv projection ----
            with (
                tc.tile_pool(name="xtp", bufs=16) as xt_pool,
                tc.tile_pool(name="projps", bufs=2, space="PSUM") as proj_ps,
            ):
                ev = 0
                for p in range(NTQ):
                    xts = []
                    for dc in range(8):
                        xtile = xt_pool.tile([128, TQ], f32r, tag="xt")
                        nc.sync.dma_start(
                            out=xtile[:],
                            in_=xt_d[128 * dc : 128 * (dc + 1), TQ * p : TQ * (p + 1)],
                        )
                        xts.append(xtile)
                    for s in range(3):
                        ps = proj_ps.tile([128, TQ], f32, tag="proj")
                        for dc in range(8):
                            g = s * 8 + dc
                            nc.tensor.matmul(
                                ps[:],
                                w_all[:, g * 128 : (g + 1) * 128],
                                xts[dc][:],
                                start=(dc == 0),
                                stop=(dc == 7),
                            )
                        u0, u1 = TQ * p, TQ * (p + 1)
                        for lo, hi, dst, t0 in SPLITS[s]:
                            a0, a1 = max(lo, u0), min(hi, u1)
                            if a0 >= a1:
                                continue
                            view = dstmap[dst][:].rearrange("p (a e) -> p a e", e=3)[
                                :, a0 - lo : a1 - lo, t0
                            ]
                            src = ps[:, a0 - u0 : a1 - u0]
                            if ev % 2 == 0:
                                nc.scalar.activation(
                                    view, src, AF.Identity, bias=b_sb[:, s : s + 1]
                                )
                            else:
                                nc.vector.tensor_scalar_add(
                                    view, src, b_sb[:, s : s + 1]
                                )
                            ev += 1

            # ---- Phase B: attention ----
            with (
                tc.tile_pool(name="scps", bufs=3, space="PSUM") as sc_ps,
                tc.tile_pool(name="ctxps", bufs=2, space="PSUM") as ctx_ps,
                tc.tile_pool(name="expp", bufs=4) as exp_pool,
                tc.tile_pool(name="ctxsb", bufs=2) as ctxs_pool,
                tc.tile_pool(name="outp", bufs=8) as out_pool,
            ):
                # A2: build v_ext = [v | 1] per head, v transposed to (tk, j)
                for c in range(NTK):
                    trp = sc_ps.tile([128, 128], f32, tag="sc")
                    nc.tensor.transpose(
                        trp[:], vT[:, 128 * c : 128 * (c + 1)], ident[:]
                    )
                    for h in (0, 1):
                        nc.vector.tensor_copy(
                            vext[h][:, 65 * c : 65 * c + 64],
                            trp[:, 64 * h : 64 * (h + 1)],
                        )
                for h in (0, 1):
                    ve_view = vext[h][:].rearrange("p (c e) -> p c e", e=65)[:, :, 64]
                    nc.vector.tensor_copy(
                        ve_view, ones_sb[:].broadcast_to([128, NTK])
                    )

                for i in range(NTQ):
                    cps = [ctx_ps.tile([65, TQ], f32, tag="ctx", name=f"ctx_{i}_{hh}") for hh in (0, 1)]
                    nvalid = 4 * i + 4
                    for g in range(nvalid // 2):
                        chunks = (2 * g, 2 * g + 1)
                        vs0 = max(0, 128 * chunks[0] - TQ * i)
                        for h in (0, 1):
                            sct = sc_ps.tile([128, 1024], f32, tag="sc")
                            for idx, c in enumerate(chunks):
                                nc.tensor.matmul(
                                    sct[:, 512 * idx : 512 * (idx + 1)],
                                    kT[64 * h : 64 * (h + 1), 128 * c : 128 * (c + 1)],
                                    qT[64 * h : 64 * (h + 1), TQ * i : TQ * (i + 1)],
                                    start=True,
                                    stop=True,
                                )
                            ext = exp_pool.tile([128, 1024], f32r, tag="exp")
                            nc.scalar.activation(
                                ext[:, vs0:1024], sct[:, vs0:1024], AF.Exp, scale=SCALE
                            )
                            for idx, c in enumerate(chunks):
                                if c >= 4 * i:  # diagonal chunk: mask tk > tq
                                    vs = 128 * c - TQ * i
                                    if 512 - vs >= 256:
                                        nc.vector.tensor_mul(
                                            ext[:, 512 * idx + vs : 512 * idx + vs + 128],
                                            ext[:, 512 * idx + vs : 512 * idx + vs + 128],
                                            mask_sb[:],
                                        )
                                    else:
                                        # pad AV to N=256: zero [vs-128, vs) too
                                        nc.vector.tensor_mul(
                                            ext[:, 512 * idx + vs - 128 : 512 * (idx + 1)],
                                            ext[:, 512 * idx + vs - 128 : 512 * (idx + 1)],
                                            mask2_sb[:],
                                        )
                            for idx, c in enumerate(chunks):
                                vs = max(0, 128 * c - TQ * i)
                                if 512 - vs < 256:
                                    vs = 512 - 256
                                nc.tensor.matmul(
                                    cps[h][:, vs:512],
                                    vext[h][:, 65 * c : 65 * c + 65],
                                    ext[:, 512 * idx + vs : 512 * (idx + 1)],
                                    start=(c == 0),
                                    stop=(c == nvalid - 1),
                                )
                    # epilogue: transpose ctx, normalize, store
                    ots = []
                    for k4 in range(4):
                        ot = out_pool.tile([128, 128], f32, tag="osb", name=f"osb_{i}_{k4}")
                        ots.append(ot)
                    for h in (0, 1):
                        cs = ctxs_pool.tile([65, TQ], f32, tag="ctxs")
                        nc.vector.tensor_copy(cs[:], cps[h][:])
                        for k4 in range(4):
                            trp = sc_ps.tile([128, 65], f32, tag="sc")
                            nc.tensor.transpose(
                                trp[:],
                                cs[:, 128 * k4 : 128 * (k4 + 1)],
                                ident[0:65, 0:65],
                            )
                            rec = out_pool.tile([128, 1], f32, tag="rec")
                            nc.vector.reciprocal(rec[:], trp[:, 64:65])
                            nc.vector.tensor_scalar_mul(
                                ots[k4][:, 64 * h : 64 * (h + 1)], trp[:, 0:64], rec[:]
                            )
                    for k4 in range(4):
                        nc.sync.dma_start(
                            out=out_d[TQ * i + 128 * k4 : TQ * i + 128 * (k4 + 1), :],
                            in_=ots[k4][:],
                        )

    nc.compile()
    return nc


def _round_f32r(x: np.ndarray) -> np.ndarray:
    """Round fp32 to fp32r (11-bit mantissa, RNE) — matches TRN2 hardware."""
    xi = np.ascontiguousarray(x).view(np.uint32)
    keep = xi & np.uint32(0xFFFFF000)
    rem = xi & np.uint32(0xFFF)
    half = np.uint32(0x800)
    lowbit = np.uint32(0x1000)
    up = keep + lowbit
    use_up = (rem > half) | ((rem == half) & ((keep & lowbit) != 0))
    return np.where(use_up, up, keep).astype(np.uint32).view(np.float32)


_NC = None


def _get_program():
    global _NC
    if _NC is None:
        _NC = _build_program()
    return _NC


def prepare_inputs(x, Wqkv, bqkv):
    x = np.asarray(x, dtype=np.float32)
    Wqkv = np.asarray(Wqkv, dtype=np.float32)
    bqkv = np.asarray(bqkv, dtype=np.float32)
    xt = _round_f32r(np.ascontiguousarray(x.reshape(T, D).T))  # (D, T)
    mask = np.triu(np.ones((128, 128), np.float32))  # keep tk <= tq
    mask2 = np.concatenate(
        [np.zeros((128, 128), np.float32), np.triu(np.ones((128, 128), np.float32))],
        axis=1,
    )
    in_maps = []
    for c in range(NCORES):
        h0, h1 = 2 * c, 2 * c + 1
        cols = np.concatenate([np.arange(HS) * NH + h0, np.arange(HS) * NH + h1])
        w_c = np.stack([Wqkv[:, s * D + cols] for s in range(3)])  # (3, 1024, 128)
        b_c = np.stack([bqkv[s * D + cols] for s in range(3)], axis=1)  # (128, 3)
        in_maps.append(
            {
                "xt": xt,
                "w": _round_f32r(np.ascontiguousarray(w_c)),
                "b": np.ascontiguousarray(b_c),
                "m": mask,
                "m2": mask2,
            }
        )
    return in_maps


def kernel(x, Wqkv, bqkv):
    nc = _get_program()
    in_maps = prepare_inputs(x, Wqkv, bqkv)
    res = run_bass_kernel_spmd(nc, in_maps, list(range(NCORES)))
    out = np.empty((1, T, D), np.float32)
    for c in range(NCORES):
        out[0, :, 128 * c : 128 * (c + 1)] = res.results[c]["out"]
    return out


# revision 13
# speedup vs baseline: 15.0821x; 15.0821x over previous
"""Causal attention (nn_CausalAttention) TRN2 Bass kernel.

Tensor-parallel over the 16 heads -> 2 heads per NeuronCore. Per core:
  - qkv projection computed transposed (col-major): M_s = W_s^T @ x^T on the
    PE, evictions scatter stride-3 into qT/kT/vT, reproducing the reference's
    raw-memory reshape(3,B,T,HS,NH) semantics (row r = 3u+s of the (3T,D)
    reinterpret is x[u] @ W[:, s*D:...]).
  - qT/kT/vT/v_ext are SPLIT into column-range sub-tiles aligned to the
    projection u-tiles (r-boundaries at multiples of 1536), so attention for
    early tq tiles overlaps the (DMA-bound) tail of the projection.
  - causal attention with scores transposed (tk on partitions), exp on
    ScalarE (scale folded, batched over GRP chunks), A@V accumulated in PSUM
    with an appended ones-column producing softmax row sums.
  - PE-transpose of ctx, DVE reciprocal normalize, contiguous DMA out.
All matmuls use float32r (fp32 rounded to 11-bit mantissa, full PE rate);
host inputs are pre-rounded to match hardware.
"""

import os
import sys

import numpy as np

for _p in ("/opt/trn_rl_repo", "/root/.axon_site/_ro/trn_rl_repo"):
    if os.path.isdir(_p) and _p not in sys.path:
        sys.path.insert(0, _p)

import concourse.bass as bass  # noqa: E402
import concourse.mybir as mybir  # noqa: E402
import concourse.tile as tile  # noqa: E402
from concourse import bacc  # noqa: E402
from concourse.bass_utils import run_bass_kernel_spmd  # noqa: E402
from concourse.masks import make_identity  # noqa: E402

f32 = mybir.dt.float32
f32r = mybir.dt.float32r
bf16 = mybir.dt.bfloat16
AF = mybir.ActivationFunctionType
ADT = bf16 if os.environ.get("KDT", "f32r") == "bf16" else f32r

T, D, NH, HS = 4096, 1024, 16, 64
SCALE = HS ** -0.5  # 0.125
NCORES = 8
TQ = 512
NTQ = T // TQ  # 8
TK = 128
NTK = T // TK  # 32
GRP = int(os.environ.get("KGRP", "3"))  # tk chunks per scores-psum group

# (u_lo, u_hi, dst, t0): M_s[:, u] for u in [u_lo,u_hi) -> dst col 3*(u-u_lo)+t0
SPLITS = {
    0: [(0, 1366, "q", 0), (1366, 2731, "k", 2), (2731, 4096, "v", 1)],
    1: [(0, 1365, "q", 1), (1365, 2731, "k", 0), (2731, 4096, "v", 2)],
    2: [(0, 1365, "q", 2), (1365, 2730, "k", 1), (2730, 4096, "v", 0)],
}

# sub-tile column ranges per destination (t-space); boundaries align with
# projection u-tile boundaries (r = 3u+s, r-boundaries multiples of 1536)
SUBS = {
    "q": [(0, 1536), (1536, 3072), (3072, 4096)],
    "k": [(0, 512), (512, 2048), (2048, 3584), (3584, 4096)],
    "v": [(0, 1024), (1024, 2560), (2560, 4096)],
}
BASE_R = {"q": 0, "k": 4096, "v": 8192}
# v_ext chunk groups matching the v sub-tiles (8, 12, 12 chunks of 128 tk)
VE_GROUPS = [(0, 8), (8, 20), (20, 32)]

# interleaved emission order: proj u-tiles / A2 groups / attention tq tiles
ORDER = [
    ("proj", 2), ("proj", 5), ("proj", 0), ("a2", 0), ("attn", 0),
    ("proj", 3), ("attn", 1), ("proj", 6), ("a2", 1), ("attn", 2),
    ("proj", 1), ("attn", 3), ("proj", 4), ("attn", 4),
    ("proj", 7), ("a2", 2), ("attn", 5), ("attn", 6), ("attn", 7),
]


def _pad3(w):
    return ((w + 2) // 3) * 3


def _sub_for(dst, t):
    for j, (lo, hi) in enumerate(SUBS[dst]):
        if lo <= t < hi:
            return j
    raise ValueError((dst, t))


class Rep:
    def __init__(self, nc, rep, consts, bigs, pools):
        self.nc = nc
        self.rep = rep
        (self.w_all, self.b_sb, self.mask_sb, self.mask2_sb, self.ident,
         self.ones_sb) = consts
        (self.qs, self.ks, self.vs, self.ves) = bigs  # sub-tile lists
        (self.xt_pool, self.sc_ps, self.ctx_ps, self.exp_pool, self.ctxs_pool,
         self.out_pool) = pools
        self.ev = 0

    def emit_proj(self, p):
        nc = self.nc
        xt_d = nc._io["xt"]
        xts = []
        for dc in range(8):
            xtile = self.xt_pool.tile(
                [128, TQ], f32r, tag="xt", name=f"xt_{self.rep}_{p}_{dc}"
            )
            nc.sync.dma_start(
                out=xtile[:],
                in_=xt_d[128 * dc : 128 * (dc + 1), TQ * p : TQ * (p + 1)],
            )
            xts.append(xtile)
        for s in range(3):
            ps = self.sc_ps.tile(
                [128, TQ], f32, tag="sc", name=f"proj_{self.rep}_{p}_{s}"
            )
            for dc in range(8):
                g = s * 8 + dc
                nc.tensor.matmul(
                    ps[:],
                    self.w_all[:, g * 128 : (g + 1) * 128],
                    xts[dc][:],
                    start=(dc == 0),
                    stop=(dc == 7),
                )
            u0, u1 = TQ * p, TQ * (p + 1)
            for lo, hi, dst, t0 in SPLITS[s]:
                a0, a1 = max(lo, u0), min(hi, u1)
                if a0 >= a1:
                    continue
                # global row r = 3u + s; sub-tile by t = r - BASE_R[dst]
                r0 = 3 * a0 + s
                t_g = r0 - BASE_R[dst]
                j = _sub_for(dst, t_g)
                sub = {"q": self.qs, "k": self.ks, "v": self.vs}[dst][j]
                t_lo = SUBS[dst][j][0]
                tl = t_g - t_lo
                a_idx, e_idx = tl // 3, tl % 3
                view = sub[:].rearrange("p (a e) -> p a e", e=3)[
                    :, a_idx : a_idx + (a1 - a0), e_idx
                ]
                src = ps[:, a0 - u0 : a1 - u0]
                if self.ev % 2 == 0:
                    nc.scalar.activation(
                        view, src, AF.Identity, bias=self.b_sb[:, s : s + 1]
                    )
                else:
                    nc.vector.tensor_scalar_add(
                        view, src, self.b_sb[:, s : s + 1]
                    )
                self.ev += 1

    def emit_a2(self, gi):
        nc = self.nc
        c0, c1 = VE_GROUPS[gi]
        vlo, vhi = SUBS["v"][gi]
        vsub = self.vs[gi]
        for c in range(c0, c1):
            toff = 128 * c - vlo
            trp = self.sc_ps.tile(
                [128, 128], f32, tag="sc", name=f"vtr_{self.rep}_{c}"
            )
            nc.tensor.transpose(
                trp[:], vsub[:, toff : toff + 128], self.ident[:]
            )
            for h in (0, 1):
                dst = self.ves[gi][h][:, 65 * (c - c0) : 65 * (c - c0) + 64]
                src = trp[:, 64 * h : 64 * (h + 1)]
                if (c + h) % 2 == 0:
                    nc.vector.tensor_copy(dst, src)
                else:
                    nc.scalar.activation(dst, src, AF.Identity)
        nch = c1 - c0
        for h in (0, 1):
            ve_view = self.ves[gi][h][:].rearrange("p (c e) -> p c e", e=65)[
                :, :, 64
            ]
            nc.vector.tensor_copy(
                ve_view, self.ones_sb[:].broadcast_to([128, nch])
            )

    def _kT(self, c):
        t = 128 * c
        j = _sub_for("k", t)
        lo = SUBS["k"][j][0]
        return self.ks[j][:, t - lo : t - lo + 128]

    def _vext(self, h, c):
        for gi, (c0, c1) in enumerate(VE_GROUPS):
            if c0 <= c < c1:
                return self.ves[gi][h][:, 65 * (c - c0) : 65 * (c - c0) + 65]
        raise ValueError(c)

    def emit_attn(self, i):
        nc = self.nc
        out_d = nc._io["out"]
        qj = _sub_for("q", TQ * i)
        qlo = SUBS["q"][qj][0]
        qsl = self.qs[qj]
        cps = [
            self.ctx_ps.tile([65, TQ], f32, tag="ctx", name=f"ctx_{self.rep}_{i}_{hh}")
            for hh in (0, 1)
        ]
        nvalid = 4 * i + 4
        ngrp = (nvalid + GRP - 1) // GRP
        for g in range(ngrp):
            chunks = tuple(range(GRP * g, min(GRP * (g + 1), nvalid)))
            glen = len(chunks)
            vs0 = max(0, 128 * chunks[0] - TQ * i)
            for h in (0, 1):
                sct = self.sc_ps.tile(
                    [128, 512 * GRP], f32, tag="sc",
                    name=f"sc_{self.rep}_{i}_{g}_{h}",
                )
                for idx, c in enumerate(chunks):
                    nc.tensor.matmul(
                        sct[:, 512 * idx : 512 * (idx + 1)],
                        self._kT(c)[64 * h : 64 * (h + 1), :],
                        qsl[64 * h : 64 * (h + 1),
                            TQ * i - qlo : TQ * (i + 1) - qlo],
                        start=True,
                        stop=True,
                    )
                ext = self.exp_pool.tile(
                    [128, 512 * GRP], ADT, tag="exp",
                    name=f"exp_{self.rep}_{i}_{g}_{h}",
                )
                # exp must also cover the zero-padded prefix of any AV matmul
                # widened to N=256 (mask2 case) so no stale SBUF reaches AV
                exp_lo = vs0
                for idx, c in enumerate(chunks):
                    vs = max(0, 128 * c - TQ * i)
                    if 512 - vs < 256:
                        exp_lo = min(exp_lo, 512 * idx + vs - 128)
                nc.scalar.activation(
                    ext[:, exp_lo : 512 * glen], sct[:, exp_lo : 512 * glen],
                    AF.Exp, scale=SCALE,
                )
                for idx, c in enumerate(chunks):
                    if c >= 4 * i:  # diagonal chunk: mask tk > tq
                        vs = 128 * c - TQ * i
                        if 512 - vs >= 256:
                            nc.vector.tensor_mul(
                                ext[:, 512 * idx + vs : 512 * idx + vs + 128],
                                ext[:, 512 * idx + vs : 512 * idx + vs + 128],
                                self.mask_sb[:],
                            )
                        else:
                            nc.vector.tensor_mul(
                                ext[:, 512 * idx + vs - 128 : 512 * (idx + 1)],
                                ext[:, 512 * idx + vs - 128 : 512 * (idx + 1)],
                                self.mask2_sb[:],
                            )
                for idx, c in enumerate(chunks):
                    vs = max(0, 128 * c - TQ * i)
                    if 512 - vs < 256:
                        vs = 512 - 256
                    nc.tensor.matmul(
                        cps[h][:, vs:512],
                        self._vext(h, c),
                        ext[:, 512 * idx + vs : 512 * (idx + 1)],
                        start=(c == 0),
                        stop=(c == nvalid - 1),
                    )
        # epilogue: transpose ctx, normalize, store
        ots = []
        for k4 in range(4):
            ot = self.out_pool.tile(
                [128, 128], f32, tag="osb", name=f"osb_{self.rep}_{i}_{k4}"
            )
            ots.append(ot)
        for h in (0, 1):
            cs = self.ctxs_pool.tile(
                [65, TQ], f32, tag="ctxs", name=f"cs_{self.rep}_{i}_{h}"
            )
            nc.vector.tensor_copy(cs[:], cps[h][:])
            for k4 in range(4):
                trp = self.sc_ps.tile(
                    [128, 65], f32, tag="sc", name=f"ctr_{self.rep}_{i}_{h}_{k4}"
                )
                nc.tensor.transpose(
                    trp[:], cs[:, 128 * k4 : 128 * (k4 + 1)],
                    self.ident[0:65, 0:65],
                )
                rec = self.out_pool.tile(
                    [128, 1], f32, tag="rec", name=f"rec_{self.rep}_{i}_{h}_{k4}"
                )
                nc.vector.reciprocal(rec[:], trp[:, 64:65])
                nc.vector.tensor_scalar_mul(
                    ots[k4][:, 64 * h : 64 * (h + 1)], trp[:, 0:64], rec[:]
                )
        for k4 in range(4):
            nc.sync.dma_start(
                out=out_d[TQ * i + 128 * k4 : TQ * i + 128 * (k4 + 1), :],
                in_=ots[k4][:],
            )

    def emit(self, parts=("proj", "a2", "attn")):
        for kind, arg in ORDER:
            if kind == "proj" and "proj" in parts:
                self.emit_proj(arg)
            elif kind == "a2" and "a2" in parts:
                self.emit_a2(arg)
            elif kind == "attn" and "attn" in parts:
                self.emit_attn(arg)


def _build_program(reps=1, trace_sim=False):
    nc = bacc.Bacc(
        "TRN2", target_bir_lowering=False, debug=False, num_devices=NCORES
    )
    nc._io = {
        "xt": nc.dram_tensor("xt", [D, T], f32r, kind="ExternalInput").ap(),
        "w": nc.dram_tensor("w", [3, D, 128], f32r, kind="ExternalInput").ap(),
        "b": nc.dram_tensor("b", [128, 3], f32, kind="ExternalInput").ap(),
        "m": nc.dram_tensor("m", [128, 128], ADT, kind="ExternalInput").ap(),
        "m2": nc.dram_tensor("m2", [128, 256], ADT, kind="ExternalInput").ap(),
        "out": nc.dram_tensor("out", [T, 128], f32, kind="ExternalOutput").ap(),
    }

    with tile.TileContext(nc, trace_sim=trace_sim) as tc:
        with (
            tc.tile_pool(name="const", bufs=1) as const_pool,
            tc.tile_pool(name="big", bufs=1) as big_pool,
            tc.tile_pool(name="xtp", bufs=16) as xt_pool,
            tc.tile_pool(
                name="scps", bufs=(3 if GRP == 2 else 2), space="PSUM"
            ) as sc_ps,
            tc.tile_pool(name="ctxps", bufs=2, space="PSUM") as ctx_ps,
            tc.tile_pool(name="expp", bufs=4) as exp_pool,
            tc.tile_pool(name="ctxsb", bufs=2) as ctxs_pool,
            tc.tile_pool(name="outp", bufs=8) as out_pool,
        ):
            w_all = const_pool.tile([128, 3 * 8 * 128], f32r, tag="w_all")
            w_view = nc._io["w"].rearrange("s (dc d) c -> d (s dc) c", d=128)
            nc.sync.dma_start(
                out=w_all[:].rearrange("d (g c) -> d g c", c=128), in_=w_view
            )
            b_sb = const_pool.tile([128, 3], f32, tag="b_sb")
            nc.sync.dma_start(out=b_sb[:], in_=nc._io["b"][:])
            mask_sb = const_pool.tile([128, 128], ADT, tag="mask")
            nc.sync.dma_start(out=mask_sb[:], in_=nc._io["m"][:])
            mask2_sb = const_pool.tile([128, 256], ADT, tag="mask2")
            nc.sync.dma_start(out=mask2_sb[:], in_=nc._io["m2"][:])
            ident = const_pool.tile([128, 128], f32, tag="ident")
            make_identity(nc, ident[:])
            ones_sb = const_pool.tile([128, 1], f32, tag="ones")
            nc.vector.memset(ones_sb[:], 1.0)

            qs = [
                big_pool.tile([128, _pad3(hi - lo)], ADT, tag=f"q{j}",
                              name=f"qT{j}")
                for j, (lo, hi) in enumerate(SUBS["q"])
            ]
            ks = [
                big_pool.tile([128, _pad3(hi - lo)], ADT, tag=f"k{j}",
                              name=f"kT{j}")
                for j, (lo, hi) in enumerate(SUBS["k"])
            ]
            vs = [
                big_pool.tile([128, _pad3(hi - lo)], f32, tag=f"v{j}",
                              name=f"vT{j}")
                for j, (lo, hi) in enumerate(SUBS["v"])
            ]
            ves = [
                [
                    big_pool.tile(
                        [128, 65 * (c1 - c0)], ADT, tag=f"ve{gi}{h}",
                        name=f"vext{gi}{h}",
                    )
                    for h in (0, 1)
                ]
                for gi, (c0, c1) in enumerate(VE_GROUPS)
            ]

            consts = (w_all, b_sb, mask_sb, mask2_sb, ident, ones_sb)
            bigs = (qs, ks, vs, ves)
            pools = (xt_pool, sc_ps, ctx_ps, exp_pool, ctxs_pool, out_pool)
            kpart = os.environ.get("KPART", "full")
            partmap = {
                "full": ("proj", "a2", "attn"),
                "attn": ("attn",),
                "proj": ("proj",),
                "proja2": ("proj", "a2"),
            }
            for rep in range(reps):
                parts = ("proj", "a2", "attn") if rep == 0 else partmap[kpart]
                Rep(nc, rep, consts, bigs, pools).emit(parts)

    nc.compile()
    return nc


def _round_f32r(x: np.ndarray) -> np.ndarray:
    """Round fp32 to fp32r (11-bit mantissa, RNE) — matches TRN2 hardware."""
    xi = np.ascontiguousarray(x).view(np.uint32)
    keep = xi & np.uint32(0xFFFFF000)
    rem = xi & np.uint32(0xFFF)
    half = np.uint32(0x800)
    lowbit = np.uint32(0x1000)
    up = keep + lowbit
    use_up = (rem > half) | ((rem == half) & ((keep & lowbit) != 0))
    return np.where(use_up, up, keep).astype(np.uint32).view(np.float32)


_NC = None


def _get_program():
    global _NC
    if _NC is None:
        _NC = _build_program(
            reps=int(os.environ.get("KREPS", "1")),
            trace_sim=bool(int(os.environ.get("KTRACE", "0"))),
        )
    return _NC


def prepare_inputs(x, Wqkv, bqkv):
    x = np.asarray(x, dtype=np.float32)
    Wqkv = np.asarray(Wqkv, dtype=np.float32)
    bqkv = np.asarray(bqkv, dtype=np.float32)
    xt = _round_f32r(np.ascontiguousarray(x.reshape(T, D).T))  # (D, T)
    mnp = mybir.dt.np(ADT)
    mask = np.triu(np.ones((128, 128), np.float32)).astype(mnp)  # keep tk<=tq
    mask2 = np.concatenate(
        [np.zeros((128, 128), np.float32), np.triu(np.ones((128, 128), np.float32))],
        axis=1,
    ).astype(mnp)
    in_maps = []
    for c in range(NCORES):
        h0, h1 = 2 * c, 2 * c + 1
        cols = np.concatenate([np.arange(HS) * NH + h0, np.arange(HS) * NH + h1])
        w_c = np.stack([Wqkv[:, s * D + cols] for s in range(3)])  # (3,1024,128)
        b_c = np.stack([bqkv[s * D + cols] for s in range(3)], axis=1)  # (128,3)
        in_maps.append(
            {
                "xt": xt,
                "w": _round_f32r(np.ascontiguousarray(w_c)),
                "b": np.ascontiguousarray(b_c),
                "m": mask,
                "m2": mask2,
            }
        )
    return in_maps


def kernel(x, Wqkv, bqkv):
    nc = _get_program()
    in_maps = prepare_inputs(x, Wqkv, bqkv)
    res = run_bass_kernel_spmd(nc, in_maps, list(range(NCORES)))
    out = np.empty((1, T, D), np.float32)
    for c in range(NCORES):
        out[0, :, 128 * c : 128 * (c + 1)] = res.results[c]["out"]
    return out


# revision 14
# speedup vs baseline: 255.4784x; 16.9392x over previous
"""Causal attention (nn_CausalAttention) TRN2 Bass kernel.

Tensor-parallel over the 16 heads -> 2 heads per NeuronCore. Per core:
  - qkv projection computed transposed (col-major): M_s = W_s^T @ x^T on the
    PE, evictions scatter stride-3 into qT/kT/vT, reproducing the reference's
    raw-memory reshape(3,B,T,HS,NH) semantics (row r = 3u+s of the (3T,D)
    reinterpret is x[u] @ W[:, s*D:...]).
  - qT/kT/vT/v_ext are SPLIT into column-range sub-tiles aligned to the
    projection u-tiles (r-boundaries at multiples of 1536), so attention for
    early tq tiles overlaps the (DMA-bound) tail of the projection.
  - causal attention with scores transposed (tk on partitions), exp on
    ScalarE (scale folded, batched over GRP chunks), A@V accumulated in PSUM
    with an appended ones-column producing softmax row sums.
  - PE-transpose of ctx, DVE reciprocal normalize, contiguous DMA out.
All matmuls use float32r (fp32 rounded to 11-bit mantissa, full PE rate);
host inputs are pre-rounded to match hardware.
"""

import os
import sys

import numpy as np

for _p in ("/opt/trn_rl_repo", "/root/.axon_site/_ro/trn_rl_repo"):
    if os.path.isdir(_p) and _p not in sys.path:
        sys.path.insert(0, _p)

import concourse.bass as bass  # noqa: E402
import concourse.mybir as mybir  # noqa: E402
import concourse.tile as tile  # noqa: E402
from concourse import bacc  # noqa: E402
from concourse.bass_utils import run_bass_kernel_spmd  # noqa: E402
from concourse.masks import make_identity  # noqa: E402

f32 = mybir.dt.float32
f32r = mybir.dt.float32r
bf16 = mybir.dt.bfloat16
AF = mybir.ActivationFunctionType
ADT = bf16 if os.environ.get("KDT", "f32r") == "bf16" else f32r

T, D, NH, HS = 4096, 1024, 16, 64
SCALE = HS ** -0.5  # 0.125
NCORES = 8
TQ = 512
NTQ = T // TQ  # 8
TK = 128
NTK = T // TK  # 32
GRP = int(os.environ.get("KGRP", "3"))  # tk chunks per scores-psum group

# (u_lo, u_hi, dst, t0): M_s[:, u] for u in [u_lo,u_hi) -> dst col 3*(u-u_lo)+t0
SPLITS = {
    0: [(0, 1366, "q", 0), (1366, 2731, "k", 2), (2731, 4096, "v", 1)],
    1: [(0, 1365, "q", 1), (1365, 2731, "k", 0), (2731, 4096, "v", 2)],
    2: [(0, 1365, "q", 2), (1365, 2730, "k", 1), (2730, 4096, "v", 0)],
}

# sub-tile column ranges per destination (t-space); boundaries align with
# projection u-tile boundaries (r = 3u+s, r-boundaries multiples of 1536)
SUBS = {
    "q": [(0, 1536), (1536, 3072), (3072, 4096)],
    "k": [(0, 512), (512, 2048), (2048, 3584), (3584, 4096)],
    "v": [(0, 1024), (1024, 2560), (2560, 4096)],
}
BASE_R = {"q": 0, "k": 4096, "v": 8192}
# v_ext chunk groups matching the v sub-tiles (8, 12, 12 chunks of 128 tk)
VE_GROUPS = [(0, 8), (8, 20), (20, 32)]

# interleaved emission order: proj u-tiles / A2 groups / attention tq tiles
ORDER = [
    ("proj", 2), ("proj", 5), ("proj", 0), ("a2", 0), ("attn", 0),
    ("proj", 3), ("attn", 1), ("proj", 6), ("a2", 1), ("attn", 2),
    ("proj", 1), ("attn", 3), ("proj", 4), ("attn", 4),
    ("proj", 7), ("a2", 2), ("attn", 5), ("attn", 6), ("attn", 7),
]


def _pad3(w):
    return ((w + 2) // 3) * 3


def _sub_for(dst, t):
    for j, (lo, hi) in enumerate(SUBS[dst]):
        if lo <= t < hi:
            return j
    raise ValueError((dst, t))


class Rep:
    def __init__(self, nc, rep, consts, bigs, pools):
        self.nc = nc
        self.rep = rep
        (self.w_all, self.b_sb, self.mask_sb, self.mask2_sb, self.ident,
         self.ones_sb) = consts
        (self.qs, self.ks, self.vs, self.ves) = bigs  # sub-tile lists
        (self.xt_pool, self.sc_ps, self.ctx_ps, self.exp_pool, self.ctxs_pool,
         self.out_pool) = pools
        self.ev = 0

    def emit_proj(self, p):
        nc = self.nc
        xt_d = nc._io["xt"]
        xts = []
        for dc in range(8):
            xtile = self.xt_pool.tile(
                [128, TQ], f32r, tag="xt", name=f"xt_{self.rep}_{p}_{dc}"
            )
            nc.sync.dma_start(
                out=xtile[:],
                in_=xt_d[128 * dc : 128 * (dc + 1), TQ * p : TQ * (p + 1)],
            )
            xts.append(xtile)
        for s in range(3):
            ps = self.sc_ps.tile(
                [128, TQ], f32, tag="sc", name=f"proj_{self.rep}_{p}_{s}"
            )
            for dc in range(8):
                g = s * 8 + dc
                nc.tensor.matmul(
                    ps[:],
                    self.w_all[:, g * 128 : (g + 1) * 128],
                    xts[dc][:],
                    start=(dc == 0),
                    stop=(dc == 7),
                )
            u0, u1 = TQ * p, TQ * (p + 1)
            for lo, hi, dst, t0 in SPLITS[s]:
                a0, a1 = max(lo, u0), min(hi, u1)
                if a0 >= a1:
                    continue
                # global row r = 3u + s; sub-tile by t = r - BASE_R[dst]
                r0 = 3 * a0 + s
                t_g = r0 - BASE_R[dst]
                j = _sub_for(dst, t_g)
                sub = {"q": self.qs, "k": self.ks, "v": self.vs}[dst][j]
                t_lo = SUBS[dst][j][0]
                tl = t_g - t_lo
                a_idx, e_idx = tl // 3, tl % 3
                view = sub[:].rearrange("p (a e) -> p a e", e=3)[
                    :, a_idx : a_idx + (a1 - a0), e_idx
                ]
                src = ps[:, a0 - u0 : a1 - u0]
                if self.ev % 2 == 0:
                    nc.scalar.activation(
                        view, src, AF.Identity, bias=self.b_sb[:, s : s + 1]
                    )
                else:
                    nc.vector.tensor_scalar_add(
                        view, src, self.b_sb[:, s : s + 1]
                    )
                self.ev += 1

    def emit_a2(self, gi):
        nc = self.nc
        c0, c1 = VE_GROUPS[gi]
        vlo, vhi = SUBS["v"][gi]
        vsub = self.vs[gi]
        for c in range(c0, c1):
            toff = 128 * c - vlo
            trp = self.sc_ps.tile(
                [128, 128], f32, tag="sc", name=f"vtr_{self.rep}_{c}"
            )
            nc.tensor.transpose(
                trp[:], vsub[:, toff : toff + 128], self.ident[:]
            )
            for h in (0, 1):
                dst = self.ves[gi][h][:, 65 * (c - c0) : 65 * (c - c0) + 64]
                src = trp[:, 64 * h : 64 * (h + 1)]
                if (c + h) % 2 == 0:
                    nc.vector.tensor_copy(dst, src)
                else:
                    nc.scalar.activation(dst, src, AF.Identity)
        nch = c1 - c0
        for h in (0, 1):
            ve_view = self.ves[gi][h][:].rearrange("p (c e) -> p c e", e=65)[
                :, :, 64
            ]
            nc.vector.tensor_copy(
                ve_view, self.ones_sb[:].broadcast_to([128, nch])
            )

    def _kT(self, c):
        t = 128 * c
        j = _sub_for("k", t)
        lo = SUBS["k"][j][0]
        return self.ks[j][:, t - lo : t - lo + 128]

    def _vext(self, h, c):
        for gi, (c0, c1) in enumerate(VE_GROUPS):
            if c0 <= c < c1:
                return self.ves[gi][h][:, 65 * (c - c0) : 65 * (c - c0) + 65]
        raise ValueError(c)

    def emit_attn(self, i):
        nc = self.nc
        out_d = nc._io["out"]
        qj = _sub_for("q", TQ * i)
        qlo = SUBS["q"][qj][0]
        qsl = self.qs[qj]
        cps = [
            self.ctx_ps.tile([65, TQ], f32, tag="ctx", name=f"ctx_{self.rep}_{i}_{hh}")
            for hh in (0, 1)
        ]
        nvalid = 4 * i + 4
        ngrp = (nvalid + GRP - 1) // GRP
        for g in range(ngrp):
            chunks = tuple(range(GRP * g, min(GRP * (g + 1), nvalid)))
            glen = len(chunks)
            vs0 = max(0, 128 * chunks[0] - TQ * i)
            for h in (0, 1):
                sct = self.sc_ps.tile(
                    [128, 512 * GRP], f32, tag="sc",
                    name=f"sc_{self.rep}_{i}_{g}_{h}",
                )
                for idx, c in enumerate(chunks):
                    nc.tensor.matmul(
                        sct[:, 512 * idx : 512 * (idx + 1)],
                        self._kT(c)[64 * h : 64 * (h + 1), :],
                        qsl[64 * h : 64 * (h + 1),
                            TQ * i - qlo : TQ * (i + 1) - qlo],
                        start=True,
                        stop=True,
                    )
                ext = self.exp_pool.tile(
                    [128, 512 * GRP], ADT, tag="exp",
                    name=f"exp_{self.rep}_{i}_{g}_{h}",
                )
                # exp must also cover the zero-padded prefix of any AV matmul
                # widened to N=256 (mask2 case) so no stale SBUF reaches AV
                exp_lo = vs0
                for idx, c in enumerate(chunks):
                    vs = max(0, 128 * c - TQ * i)
                    if 512 - vs < 256:
                        exp_lo = min(exp_lo, 512 * idx + vs - 128)
                nc.scalar.activation(
                    ext[:, exp_lo : 512 * glen], sct[:, exp_lo : 512 * glen],
                    AF.Exp, scale=SCALE,
                )
                for idx, c in enumerate(chunks):
                    if c >= 4 * i:  # diagonal chunk: mask tk > tq
                        vs = 128 * c - TQ * i
                        if 512 - vs >= 256:
                            nc.vector.tensor_mul(
                                ext[:, 512 * idx + vs : 512 * idx + vs + 128],
                                ext[:, 512 * idx + vs : 512 * idx + vs + 128],
                                self.mask_sb[:],
                            )
                        else:
                            nc.vector.tensor_mul(
                                ext[:, 512 * idx + vs - 128 : 512 * (idx + 1)],
                                ext[:, 512 * idx + vs - 128 : 512 * (idx + 1)],
                                self.mask2_sb[:],
                            )
                for idx, c in enumerate(chunks):
                    vs = max(0, 128 * c - TQ * i)
                    if 512 - vs < 256:
                        vs = 512 - 256
                    nc.tensor.matmul(
                        cps[h][:, vs:512],
                        self._vext(h, c),
                        ext[:, 512 * idx + vs : 512 * (idx + 1)],
                        start=(c == 0),
                        stop=(c == nvalid - 1),
                    )
        # epilogue: transpose ctx, normalize, store
        ots = []
        for k4 in range(4):
            ot = self.out_pool.tile(
                [128, 128], f32, tag="osb", name=f"osb_{self.rep}_{i}_{k4}"
            )
            ots.append(ot)
        for h in (0, 1):
            cs = self.ctxs_pool.tile(
                [65, TQ], f32, tag="ctxs", name=f"cs_{self.rep}_{i}_{h}"
            )
            nc.vector.tensor_copy(cs[:], cps[h][:])
            for k4 in range(4):
                trp = self.sc_ps.tile(
                    [128, 65], f32, tag="sc", name=f"ctr_{self.rep}_{i}_{h}_{k4}"
                )
                nc.tensor.transpose(
                    trp[:], cs[:, 128 * k4 : 128 * (k4 + 1)],
                    self.ident[0:65, 0:65],
                )
                rec = self.out_pool.tile(
                    [128, 1], f32, tag="rec", name=f"rec_{self.rep}_{i}_{h}_{k4}"
                )
                nc.vector.reciprocal(rec[:], trp[:, 64:65])
                nc.vector.tensor_scalar_mul(
                    ots[k4][:, 64 * h : 64 * (h + 1)], trp[:, 0:64], rec[:]
                )
        for k4 in range(4):
            nc.sync.dma_start(
                out=out_d[TQ * i + 128 * k4 : TQ * i + 128 * (k4 + 1), :],
                in_=ots[k4][:],
            )

    def emit(self, parts=("proj", "a2", "attn")):
        for kind, arg in ORDER:
            if kind == "proj" and "proj" in parts:
                self.emit_proj(arg)
            elif kind == "a2" and "a2" in parts:
                self.emit_a2(arg)
            elif kind == "attn" and "attn" in parts:
                self.emit_attn(arg)


def _build_program(reps=1, trace_sim=False):
    nc = bacc.Bacc(
        "TRN2", target_bir_lowering=False, debug=False, num_devices=NCORES
    )
    nc._io = {
        "xt": nc.dram_tensor("xt", [D, T], f32r, kind="ExternalInput").ap(),
        "w": nc.dram_tensor("w", [3, D, 128], f32r, kind="ExternalInput").ap(),
        "b": nc.dram_tensor("b", [128, 3], f32, kind="ExternalInput").ap(),
        "m": nc.dram_tensor("m", [128, 128], ADT, kind="ExternalInput").ap(),
        "m2": nc.dram_tensor("m2", [128, 256], ADT, kind="ExternalInput").ap(),
        "out": nc.dram_tensor("out", [T, 128], f32, kind="ExternalOutput").ap(),
    }

    with tile.TileContext(nc, trace_sim=trace_sim) as tc:
        with (
            tc.tile_pool(name="const", bufs=1) as const_pool,
            tc.tile_pool(name="big", bufs=1) as big_pool,
            tc.tile_pool(name="xtp", bufs=24) as xt_pool,
            tc.tile_pool(
                name="scps", bufs=(3 if GRP == 2 else 2), space="PSUM"
            ) as sc_ps,
            tc.tile_pool(name="ctxps", bufs=2, space="PSUM") as ctx_ps,
            tc.tile_pool(name="expp", bufs=6) as exp_pool,
            tc.tile_pool(name="ctxsb", bufs=4) as ctxs_pool,
            tc.tile_pool(name="outp", bufs=8) as out_pool,
        ):
            w_all = const_pool.tile([128, 3 * 8 * 128], f32r, tag="w_all")
            w_view = nc._io["w"].rearrange("s (dc d) c -> d (s dc) c", d=128)
            nc.sync.dma_start(
                out=w_all[:].rearrange("d (g c) -> d g c", c=128), in_=w_view
            )
            b_sb = const_pool.tile([128, 3], f32, tag="b_sb")
            nc.sync.dma_start(out=b_sb[:], in_=nc._io["b"][:])
            mask_sb = const_pool.tile([128, 128], ADT, tag="mask")
            nc.sync.dma_start(out=mask_sb[:], in_=nc._io["m"][:])
            mask2_sb = const_pool.tile([128, 256], ADT, tag="mask2")
            nc.sync.dma_start(out=mask2_sb[:], in_=nc._io["m2"][:])
            ident = const_pool.tile([128, 128], f32, tag="ident")
            make_identity(nc, ident[:])
            ones_sb = const_pool.tile([128, 1], f32, tag="ones")
            nc.vector.memset(ones_sb[:], 1.0)

            qs = [
                big_pool.tile([128, _pad3(hi - lo)], ADT, tag=f"q{j}",
                              name=f"qT{j}")
                for j, (lo, hi) in enumerate(SUBS["q"])
            ]
            ks = [
                big_pool.tile([128, _pad3(hi - lo)], ADT, tag=f"k{j}",
                              name=f"kT{j}")
                for j, (lo, hi) in enumerate(SUBS["k"])
            ]
            vs = [
                big_pool.tile([128, _pad3(hi - lo)], f32, tag=f"v{j}",
                              name=f"vT{j}")
                for j, (lo, hi) in enumerate(SUBS["v"])
            ]
            ves = [
                [
                    big_pool.tile(
                        [128, 65 * (c1 - c0)], ADT, tag=f"ve{gi}{h}",
                        name=f"vext{gi}{h}",
                    )
                    for h in (0, 1)
                ]
                for gi, (c0, c1) in enumerate(VE_GROUPS)
            ]

            consts = (w_all, b_sb, mask_sb, mask2_sb, ident, ones_sb)
            bigs = (qs, ks, vs, ves)
            pools = (xt_pool, sc_ps, ctx_ps, exp_pool, ctxs_pool, out_pool)
            kpart = os.environ.get("KPART", "full")
            partmap = {
                "full": ("proj", "a2", "attn"),
                "attn": ("attn",),
                "proj": ("proj",),
                "proja2": ("proj", "a2"),
            }
            for rep in range(reps):
                parts = ("proj", "a2", "attn") if rep == 0 else partmap[kpart]
                Rep(nc, rep, consts, bigs, pools).emit(parts)

    nc.compile()
    return nc


def _round_f32r(x: np.ndarray) -> np.ndarray:
    """Round fp32 to fp32r (11-bit mantissa, RNE) — matches TRN2 hardware."""
    xi = np.ascontiguousarray(x).view(np.uint32)
    keep = xi & np.uint32(0xFFFFF000)
    rem = xi & np.uint32(0xFFF)
    half = np.uint32(0x800)
    lowbit = np.uint32(0x1000)
    up = keep + lowbit
    use_up = (rem > half) | ((rem == half) & ((keep & lowbit) != 0))
    return np.where(use_up, up, keep).astype(np.uint32).view(np.float32)


_NC = None


def _get_program():
    global _NC
    if _NC is None:
        _NC = _build_program(
            reps=int(os.environ.get("KREPS", "1")),
            trace_sim=bool(int(os.environ.get("KTRACE", "0"))),
        )
    return _NC


def prepare_inputs(x, Wqkv, bqkv):
    x = np.asarray(x, dtype=np.float32)
    Wqkv = np.asarray(Wqkv, dtype=np.float32)
    bqkv = np.asarray(bqkv, dtype=np.float32)
    xt = _round_f32r(np.ascontiguousarray(x.reshape(T, D).T))  # (D, T)
    mnp = mybir.dt.np(ADT)
    mask = np.triu(np.ones((128, 128), np.float32)).astype(mnp)  # keep tk<=tq
    mask2 = np.concatenate(
        [np.zeros((128, 128), np.float32), np.triu(np.ones((128, 128), np.float32))],
        axis=1,
    ).astype(mnp)
    in_maps = []
    for c in range(NCORES):
        h0, h1 = 2 * c, 2 * c + 1
        cols = np.concatenate([np.arange(HS) * NH + h0, np.arange(HS) * NH + h1])
        w_c = np.stack([Wqkv[:, s * D + cols] for s in range(3)])  # (3,1024,128)
        b_c = np.stack([bqkv[s * D + cols] for s in range(3)], axis=1)  # (128,3)
        in_maps.append(
            {
                "xt": xt,
                "w": _round_f32r(np.ascontiguousarray(w_c)),
                "b": np.ascontiguousarray(b_c),
                "m": mask,
                "m2": mask2,
            }
        )
    return in_maps


def kernel(x, Wqkv, bqkv):
    nc = _get_program()
    in_maps = prepare_inputs(x, Wqkv, bqkv)
    res = run_bass_kernel_spmd(nc, in_maps, list(range(NCORES)))
    out = np.empty((1, T, D), np.float32)
    for c in range(NCORES):
        out[0, :, 128 * c : 128 * (c + 1)] = res.results[c]["out"]
    return out
